# revision 5
# baseline (speedup 1.0000x reference)
"""Trainium2 Bass kernel for nn_CSNN (4x conv3x3->BN->LIF->maxpool + FC->LIF).

Sharding: 8 cores = 4 batch x 2 H-halves. Halo handled by recompute (no
collectives). Bottom-half cores get V-flipped inputs + dy-flipped weights so
all cores run the identical SPMD program; host unflips outputs.

Math transform (validated bit-level against the reference in numpy):
  - BN folded into conv weights/bias on host.
  - LIF charge v' = 0.5*v + 0.5*x  computed as ACT: vh = 0.5*PSUM + bias_act,
    where PSUM = conv_taps + 2*I @ u_prev (state injected via TensorE).
  - mask m' = (vh < 1)*0.5 on DVE; state u = vh*m' (hard reset + decay fold).
  - maxpool(spikes) == 1 - 2*minpool(m'); the affine spike transform is folded
    into the next conv: taps use -2*w, bias_act gains 0.5*rowsum(w).
  - everything bf16 on-chip (validated: final output exactly matches fp32 ref).
FC + final LIF run on host over the gathered block-4 spikes (0.005% of FLOPs).
"""
import numpy as np
import ml_dtypes

import concourse.bass as bass
import concourse.mybir as mybir
import concourse.tile as tile
from concourse.bass_utils import run_bass_kernel_spmd

bf16 = ml_dtypes.bfloat16
FP32 = mybir.dt.float32
BF16 = mybir.dt.bfloat16

T, B, CH = 16, 4, 128
EPS = 1e-5

# per-block geometry (identical on every core thanks to the flip trick)
R = [78, 38, 18, 8]            # conv-out rows computed per core
W = [130, 66, 34, 18]          # conv-out width incl 2 border cols
MPR = [40, 20, 10]             # mp tile rows (1 pad row + pooled rows)
MPW = [66, 34, 18]             # mp tile cols (pooled cols + 2 border)
PX = [r * w for r, w in zip(R, W)]          # 10140, 2508, 612, 144
MPSZ = [1 + r * w + 1 for r, w in zip(MPR, MPW)]   # flat + slack elems


def _ntiles(px):
    out, p = [], 0
    while p < px:
        n = min(512, px - p)
        if 0 < px - p - n < 64 and n == 512:   # avoid tiny tail tiles
            n = (px - p + 1) // 2
        out.append((p, n))
        p += n
    return out


TILES = [_ntiles(px) for px in PX]  # conv1: 507-ish x20, etc.


def _build_program():
    nc = bass.Bass('TRN2', target_bir_lowering=False, debug=False)
    xpat = nc.declare_dram_parameter("xpat", [T, 18, PX[0]], BF16, isOutput=False)
    w1 = nc.declare_dram_parameter("w1", [18, 128], BF16, isOutput=False)
    wk_ext = [nc.declare_dram_parameter(f"w{k}", [128, 9, 128], BF16,
                                        isOutput=False) for k in (2, 3, 4)]
    ident = nc.declare_dram_parameter("ident", [128, 128], BF16, isOutput=False)
    b_ext = [nc.declare_dram_parameter(f"b{k}", [128, 1], FP32, isOutput=False)
             for k in (1, 2, 3, 4)]
    out4 = nc.declare_dram_parameter("out4", [T, 128, 32], BF16, isOutput=True)

    with tile.TileContext(nc) as tc:
        with tc.tile_pool(name="const", bufs=1) as cp, \
             tc.tile_pool(name="state", bufs=1) as st, \
             tc.tile_pool(name="pat", bufs=2) as patp, \
             tc.tile_pool(name="vhp", bufs=1) as vhp, \
             tc.tile_pool(name="mw", bufs=1) as mwp, \
             tc.tile_pool(name="tmp", bufs=1) as tmpp, \
             tc.tile_pool(name="outp", bufs=2) as outp, \
             tc.tile_pool(name="ps", bufs=8, space="PSUM") as ps:

            # ---- constants ----
            w1t = cp.tile([18, 128], BF16)
            nc.sync.dma_start(out=w1t, in_=w1[:])
            wkt = []
            for k in range(3):
                wt = cp.tile([128, 9, 128], BF16, name=f"wk{k}", tag=f"wk{k}")
                nc.sync.dma_start(out=wt, in_=wk_ext[k][:])
                wkt.append(wt)
            idt = cp.tile([128, 128], BF16)
            nc.sync.dma_start(out=idt, in_=ident[:])
            bt = []
            for k in range(4):
                b = cp.tile([128, 1], FP32, name=f"bias{k}", tag=f"bias{k}")
                nc.sync.dma_start(out=b, in_=b_ext[k][:])
                bt.append(b)

            # ---- persistent state ----
            u = [st.tile([128, PX[k]], BF16, name=f"u{k}", tag=f"u{k}") for k in range(4)]
            mp = [st.tile([128, MPSZ[k]], BF16, name=f"mp{k}", tag=f"mp{k}") for k in range(3)]
            for t_ in mp:
                nc.vector.memset(t_, 0.5)

            for t in range(T):
                # ======== block 1: conv from host-built im2col patches ====
                pat = patp.tile([18, PX[0]], BF16)
                nc.sync.dma_start(out=pat, in_=xpat[t])
                vh1 = vhp.tile([128, PX[0]], BF16, name="vh1", tag="vh1")
                for (p0, n) in TILES[0]:
                    acc = ps.tile([128, n], FP32, name="psum", tag="psum")
                    nc.tensor.matmul(acc, w1t, pat[:, p0:p0 + n],
                                     start=True, stop=(t == 0))
                    if t > 0:
                        nc.tensor.matmul(acc, idt, u[0][:, p0:p0 + n],
                                         start=False, stop=True)
                    nc.scalar.activation(vh1[:, p0:p0 + n], acc,
                                         mybir.ActivationFunctionType.Identity,
                                         bias=bt[0], scale=0.5)
                self_vh = [vh1]

                # ======== blocks 2..4 ====================================
                for k in range(1, 4):
                    vhk = vhp.tile([128, PX[k]], BF16, name=f"vh{k}", tag=f"vh{k}")
                    rhs = mp[k - 1]
                    wk = wkt[k - 1]
                    for (p0, n) in TILES[k]:
                        acc = ps.tile([128, n], FP32, name="psum", tag="psum")
                        for tap in range(9):
                            dy, dx = tap // 3 - 1, tap % 3 - 1
                            s = 1 + (dy + 1) * MPW[k - 1] + dx + p0
                            nc.tensor.matmul(acc, wk[:, tap], rhs[:, s:s + n],
                                             start=(tap == 0),
                                             stop=(tap == 8 and t == 0))
                        if t > 0:
                            nc.tensor.matmul(acc, idt, u[k][:, p0:p0 + n],
                                             start=False, stop=True)
                        nc.scalar.activation(vhk[:, p0:p0 + n], acc,
                                             mybir.ActivationFunctionType.Identity,
                                             bias=bt[k], scale=0.5)
                    self_vh.append(vhk)

                # ======== LIF mask/reset + pool per block ================
                for k in range(4):
                    vhk = self_vh[k]
                    mk = mwp.tile([128, PX[k]], BF16, name=f"m{k}", tag=f"m{k}")
                    nc.vector.tensor_scalar(mk, vhk, 1.0, 0.5,
                                            mybir.AluOpType.is_lt,
                                            mybir.AluOpType.mult)
                    nc.vector.tensor_tensor(u[k], vhk, mk, mybir.AluOpType.mult)
                    rows, wdt = R[k], W[k]
                    pw = (wdt - 2) // 2
                    m3 = mk.rearrange("p (r w) -> p r w", w=wdt)
                    mv = m3[:, :, 1:1 + 2 * pw].rearrange(
                        "p r (a two) -> p r a two", two=2)
                    mn1 = tmpp.tile([128, rows * pw], BF16, name=f"mn{k}", tag=f"mn{k}")
                    n1v = mn1.rearrange("p (r a) -> p r a", a=pw)
                    nc.vector.tensor_tensor(n1v, mv[:, :, :, 0], mv[:, :, :, 1],
                                            mybir.AluOpType.min)
                    n2v = mn1.rearrange("p (r two a) -> p r two a", two=2, a=pw)
                    if k < 3:
                        mpv = mp[k][:, 1:1 + MPR[k] * MPW[k]].rearrange(
                            "p (r w) -> p r w", w=MPW[k])
                        dst = mpv[:, 1:1 + rows // 2, 1:1 + pw]
                        nc.vector.tensor_tensor(dst, n2v[:, :, 0, :],
                                                n2v[:, :, 1, :],
                                                mybir.AluOpType.min)
                    else:
                        o4 = outp.tile([128, 32], BF16, name="o4", tag="o4")
                        nc.vector.tensor_tensor(o4, n2v[:, :, 0, :],
                                                n2v[:, :, 1, :],
                                                mybir.AluOpType.min)
                        nc.sync.dma_start(out=out4[t], in_=o4)

    _split_multiwaits(nc)
    return nc


def _split_multiwaits(nc):
    """This walrus build supports only ONE sync-wait per instruction; hoist
    extras into single-wait NoOps inserted immediately before, same engine."""
    for f in nc.m.functions:
        for bb in f.blocks:
            new = []
            for inst in bb.instructions:
                si = inst.sync_info
                if si is not None and si.on_wait and len(si.on_wait) > 1:
                    waits = list(si.on_wait)
                    for j, w in enumerate(waits[:-1]):
                        new.append(mybir.InstNoOp(
                            name=f"{inst.name}-w{j}", engine=inst.engine,
                            bass_nofuse=True,
                            sync_info=mybir.SyncInfo(on_wait=[w], on_update=[])))
                    inst.sync_info = mybir.SyncInfo(
                        on_wait=[waits[-1]], on_update=list(si.on_update))
                new.append(inst)
            bb.instructions = new


def _prep_core(inputs, b, half):
    """Host-side per-core input prep (numpy)."""
    x = np.asarray(inputs['x'])[:, b]                     # [T,2,128,128]
    if half == 1:
        x = x[:, :, ::-1, :]
    xp = np.zeros((T, 2, 82, 132), np.float32)
    xp[:, :, 2:82, 2:130] = x[:, :, 0:80, :]
    xp = xp.astype(bf16)
    pat = np.empty((T, 18, PX[0]), bf16)
    for tap in range(9):
        dy, dx = tap // 3 - 1, tap % 3 - 1
        sl = xp[:, :, 2 + dy:80 + dy, 1 + dx:131 + dx]    # [T,2,78,130]
        pat[:, 2 * tap] = sl[:, 0].reshape(T, PX[0])
        pat[:, 2 * tap + 1] = sl[:, 1].reshape(T, PX[0])

    im = {"xpat": pat, "ident": (2.0 * np.eye(128)).astype(bf16)}
    for i in range(1, 5):
        w = np.asarray(inputs[f'w{i}']).astype(np.float32)
        g = np.asarray(inputs[f'g{i}']).astype(np.float32)
        bb_ = np.asarray(inputs[f'b{i}']).astype(np.float32)
        m = np.asarray(inputs[f'm{i}']).astype(np.float32)
        v = np.asarray(inputs[f'v{i}']).astype(np.float32)
        inv = g / np.sqrt(v + EPS)
        wf = w * inv[:, None, None, None]
        bnb = bb_ - m * inv
        if half == 1:
            wf = wf[:, :, ::-1, :]
        if i == 1:
            lhsT = np.empty((18, 128), bf16)
            for tap in range(9):
                dy, dx = tap // 3, tap % 3
                for c in range(2):
                    lhsT[2 * tap + c] = wf[:, c, dy, dx].astype(bf16)
            im["w1"] = lhsT
            im["b1"] = (0.5 * bnb).astype(np.float32).reshape(128, 1)
        else:
            lhsT = np.empty((128, 9, 128), bf16)
            for tap in range(9):
                dy, dx = tap // 3, tap % 3
                lhsT[:, tap] = (-2.0 * wf[:, :, dy, dx].T).astype(bf16)
            im[f"w{i}"] = lhsT
            rowsum = wf.sum(axis=(1, 2, 3))
            im[f"b{i}"] = (0.5 * (rowsum + bnb)).astype(np.float32).reshape(128, 1)
    return im


_CACHE = {}


def kernel(**inputs):
    if "nc" not in _CACHE:
        _CACHE["nc"] = _build_program()
    nc = _CACHE["nc"]

    in_maps = [_prep_core(inputs, c % B, c // B) for c in range(8)]
    res = run_bass_kernel_spmd(nc, in_maps, list(range(8)))

    s4 = np.zeros((T, B, 128, 8, 8), np.float32)
    for c in range(8):
        b, half = c % B, c // B
        mp4 = np.asarray(res.results[c]["out4"]).astype(np.float32)
        s = (1.0 - 2.0 * mp4).reshape(T, 128, 4, 8)
        if half == 0:
            s4[:, b, :, 0:4, :] = s
        else:
            s4[:, b, :, 4:8, :] = s[:, :, ::-1, :]

    wfc = np.asarray(inputs['wfc']).astype(np.float32)
    bfc = np.asarray(inputs['bfc']).astype(np.float32)
    z = s4.reshape(T, B, -1) @ wfc.T + bfc
    v = np.zeros_like(z[0])
    outs = []
    for t in range(T):
        v = v + (z[t] - v) / 2.0
        s = (v >= 1.0).astype(np.float32)
        v = v * (1.0 - s)
        outs.append(s)
    return np.stack(outs).astype(np.float32)
